# revision 42
# baseline (speedup 1.0000x reference)
"""GAT (3-layer, multi-head) forward on 8 Trainium2 NeuronCores.

Strategy (graph/data parallel, per sharding hint):
- Nodes sharded in contiguous 1280-node blocks (10 tiles of 128) per core;
  edges partitioned by destination, then split into two classes by the
  source node's position within its owner core's shard (first/second 640
  rows), sorted by src within each (dst-tile, class) for HBM locality.
- Per layer: node-phase projection (PE matmul), table rows
  [h | al_src hi | al_src lo | pad] staged to HBM in two halves; two
  AllGathers (A = every core's first 640 rows, B = second 640) so the
  class-A edge compute overlaps the AllGather-B wire time. Next layer's
  node projection is interleaved into the previous edge phase per-tile,
  so the AllGathers trigger as early as possible.
- Edge phase per (dst-tile, class): dma_gather of source rows (the only
  per-edge data movement), al_dst expanded edge-wise via one-hot^T matmul
  on PE (one-hots stored fp8, exact), attention weights exp(leakyrelu(z))
  on ACT, applied to messages split across DVE (low heads, one broadcast
  op) and ACT (high heads, per-chunk scale), aggregation + softmax
  denominators via one-hot matmul into PSUM, partials combined in SBUF,
  then normalize/bias/ELU/residual.
- Graph mean-pool via one-hot matmul accumulated over tiles, classifier
  matmul applied to the per-core partial, tiny [64,16] AllReduce, then
  log_softmax computed redundantly on every core.
"""
import sys

sys.path.insert(0, '/opt/trn_rl_repo')

import numpy as np

N_CORES = 8
N, E, F_IN, HID, H1, H3, NC_CLS, G = 10000, 160000, 256, 128, 5, 3, 10, 64
NEG_SLOPE = 0.2
SHARD = 1280            # nodes per core (10 tiles of 128); core 7 partially padded
NTILE = 10              # dst tiles per core
HSH = 640               # half-shard rows per AllGather piece
NPAD = SHARD * N_CORES  # 10240 padded node count
ROW = 768               # L1/L2 table row elems (bf16): [h 640 | al_s hi 5 | lo 5 | pad]
ROW3 = 512              # L3 table row elems: [h 384 | al hi 3 | al lo 3 | pad]
MAXH = 5                # max chunks per gather unit


def _blockdiag_a(a):
    Hh, C = a.shape
    out = np.zeros((Hh * C, Hh), np.float32)
    for h in range(Hh):
        out[h * C:(h + 1) * C, h] = a[h]
    return out


def _build_host_data(x, edge_index, batch, weights):
    """All index preprocessing + per-core constant inputs."""
    import ml_dtypes
    bf16 = ml_dtypes.bfloat16
    f8 = ml_dtypes.float8_e4m3

    src = np.concatenate([edge_index[0], np.arange(N)]).astype(np.int64)
    dst = np.concatenate([edge_index[1], np.arange(N)]).astype(np.int64)

    core_of = dst // SHARD
    tile_of = (dst % SHARD) // 128
    cls_of = ((src % SHARD) >= HSH).astype(np.int64)  # 0=A, 1=B

    counts = np.zeros((N_CORES, NTILE, 2), np.int64)
    for k in range(N_CORES):
        for t in range(NTILE):
            for cl in range(2):
                counts[k, t, cl] = np.count_nonzero(
                    (core_of == k) & (tile_of == t) & (cls_of == cl))
    nch = [[int(np.ceil(counts[:, t, cl].max() / 128)) for t in range(NTILE)]
           for cl in range(2)]
    assert all(n > 0 for n in nch[0]) and all(n > 0 for n in nch[1])
    totcA, totcB = sum(nch[0]), sum(nch[1])
    totc = totcA + totcB

    # remapped gather index: class A -> 640*k + r, class B -> 640*k + r - 640
    remap = HSH * (src // SHARD) + (src % SHARD) - cls_of * HSH

    per_core = []
    for k in range(N_CORES):
        srcs = np.zeros(totc * 128, np.int16)
        S = np.zeros((128, totc, 128), np.float32)
        base = 0
        nreal = {}
        for cl in range(2):
            for t in range(NTILE):
                m = (core_of == k) & (tile_of == t) & (cls_of == cl)
                mi = np.nonzero(m)[0]
                mi = mi[np.argsort(src[mi], kind='stable')]  # src-sorted: HBM locality
                n = len(mi)
                nreal[(cl, t)] = n
                cap = nch[cl][t] * 128
                sl = np.zeros(cap, np.int16)  # pads gather row 0 (S col is zero)
                sl[:n] = remap[mi].astype(np.int16)
                srcs[base * 128:(base + nch[cl][t]) * 128] = sl
                dloc = (dst[mi] % SHARD) % 128
                e = np.arange(n)
                S[e % 128, base + e // 128, dloc] = 1.0
                base += nch[cl][t]
        ST = np.ascontiguousarray(S.transpose(2, 1, 0))  # [128 d, totc, 128 e]

        def wrap_idx(flat):
            n_ = len(flat)
            cols = n_ // 16
            t_ = np.zeros((128, cols), np.int16)
            v = flat.reshape(cols, 16).T
            for g_ in range(8):
                t_[g_ * 16:(g_ + 1) * 16, :] = v
            return t_

        pool = np.zeros((128, NTILE, G), np.float32)
        for t in range(NTILE):
            gn = SHARD * k + 128 * t + np.arange(128)
            ok = gn < N
            pool[ok, t, batch[gn[ok]]] = 1.0

        per_core.append(dict(
            src_idx=wrap_idx(srcs),
            S=S.astype(f8), ST=ST.astype(f8),
            pool=pool.astype(bf16),
        ))

    cnt = np.bincount(batch, minlength=G).astype(np.float32)
    inv_cnt = (1.0 / np.maximum(cnt, 1.0)).astype(np.float32)

    W1, a1s, a1d, b1, W2, a2s, a2d, b2, W3, a3s, a3d, b3, Wc, bc = weights

    def ext(W, as_, ad_, width):
        A_s = W @ _blockdiag_a(as_)
        A_d = W @ _blockdiag_a(ad_)
        out = np.zeros((W.shape[0], width), np.float32)
        out[:, :W.shape[1]] = W
        out[:, W.shape[1]:W.shape[1] + as_.shape[0]] = A_s
        out[:, W.shape[1] + as_.shape[0]:W.shape[1] + 2 * as_.shape[0]] = A_d
        return out

    W1e = ext(W1, a1s, a1d, 656).astype(bf16)   # cols: 640 h, 640:645 als, 645:650 ald
    W2e = ext(W2, a2s, a2d, 656).astype(bf16)
    W3e = ext(W3, a3s, a3d, 392).astype(bf16)   # 384 h, 384:387 als, 387:390 ald

    xt_full = np.zeros((F_IN, NPAD), np.float32)
    xt_full[:, :N] = x.T
    xt_full = xt_full.astype(bf16)

    consts = dict(
        W1e=W1e, W2e=W2e, W3e=W3e,
        b1r=np.tile(b1[None, :], (128, 1)).astype(np.float32),
        b2r=np.tile(b2[None, :], (128, 1)).astype(np.float32),
        b3r=np.tile(b3[None, :], (128, 1)).astype(np.float32),
        Wc=np.concatenate([Wc, np.zeros((HID, 6), np.float32)], 1).astype(bf16),
        bcr=np.tile(bc[None, :], (G, 1)).astype(np.float32),
        inv_cnt=inv_cnt.reshape(G, 1),
    )
    for k in range(N_CORES):
        xt_loc = np.asarray(xt_full[:, SHARD * k:SHARD * (k + 1)])
        per_core[k]['xt_loc'] = np.ascontiguousarray(xt_loc)
        per_core[k].update(consts)
    return per_core, nch, totc


def _build_bass(nch, totc):
    import concourse.bass as bass
    import concourse.mybir as mybir
    import concourse.tile as tile
    from concourse import bacc

    bf = mybir.dt.bfloat16
    f8 = mybir.dt.float8e4
    f32 = mybir.dt.float32
    AT = mybir.ActivationFunctionType
    OP = mybir.AluOpType
    RG = [list(range(N_CORES))]

    totcA = sum(nch[0])

    nc = bacc.Bacc("TRN2", target_bir_lowering=False, debug=False,
                   num_devices=N_CORES, num_swdge_queues=2)

    # ---- inputs -----------------------------------------------------------
    d_xtl = nc.dram_tensor("xt_loc", [F_IN, SHARD], bf, kind="ExternalInput")
    d_W1e = nc.dram_tensor("W1e", [F_IN, 656], bf, kind="ExternalInput")
    d_W2e = nc.dram_tensor("W2e", [640, 656], bf, kind="ExternalInput")
    d_W3e = nc.dram_tensor("W3e", [640, 392], bf, kind="ExternalInput")
    d_b1 = nc.dram_tensor("b1r", [128, 640], f32, kind="ExternalInput")
    d_b2 = nc.dram_tensor("b2r", [128, 640], f32, kind="ExternalInput")
    d_b3 = nc.dram_tensor("b3r", [128, 128], f32, kind="ExternalInput")
    d_Wc = nc.dram_tensor("Wc", [HID, 16], bf, kind="ExternalInput")
    d_bcr = nc.dram_tensor("bcr", [G, NC_CLS], f32, kind="ExternalInput")
    d_icnt = nc.dram_tensor("inv_cnt", [G, 1], f32, kind="ExternalInput")
    d_S = nc.dram_tensor("S", [128, totc, 128], f8, kind="ExternalInput")
    d_ST = nc.dram_tensor("ST", [128, totc, 128], f8, kind="ExternalInput")
    d_idx = nc.dram_tensor("src_idx", [128, totc * 8], mybir.dt.int16, kind="ExternalInput")
    d_pool = nc.dram_tensor("pool", [128, NTILE, G], bf, kind="ExternalInput")
    d_out = nc.dram_tensor("out", [2, G, NC_CLS], f32, kind="ExternalOutput")

    H = {1: H1, 2: H1, 3: H3}
    HW = {1: 640, 2: 640, 3: 384}       # h width per layer
    RW = {1: ROW, 2: ROW, 3: ROW3}      # table row width per layer
    DVH = {1: 3, 2: 3, 3: 2}            # heads multiplied on DVE (rest on ACT)
    cbase = [[0], [0]]
    for cl in range(2):
        for t in range(NTILE):
            cbase[cl].append(cbase[cl][-1] + nch[cl][t])

    with tile.TileContext(nc) as tc:
        with (
            tc.tile_pool(name="const", bufs=1) as cpool,
            tc.tile_pool(name="mm", bufs=4) as mpool,
            tc.tile_pool(name="gath", bufs=5) as gpool,
            tc.tile_pool(name="gath3", bufs=4) as gpool3,
            tc.tile_pool(name="small", bufs=2) as spool,
            tc.tile_pool(name="psA", bufs=3, space="PSUM") as psA,
            tc.tile_pool(name="psZ", bufs=1, space="PSUM") as psZ,
            tc.tile_pool(name="psP", bufs=1, space="PSUM") as psP,
            tc.tile_pool(name="dram", bufs=1, space="DRAM") as dpool,
        ):
            # ---- resident constants --------------------------------------
            S = cpool.tile([128, totc, 128], f8)
            ST = cpool.tile([128, totc, 128], f8)
            IDX = cpool.tile([128, totc * 8], mybir.dt.int16)
            W1e = cpool.tile([128, 2, 656], bf)
            W2e = cpool.tile([128, 5, 656], bf)
            W3e = cpool.tile([128, 5, 392], bf)
            B1 = cpool.tile([128, 640], f32)
            B2 = cpool.tile([128, 640], f32)
            B3 = cpool.tile([128, 128], f32)
            WC = cpool.tile([128, 16], bf)
            BC = cpool.tile([G, NC_CLS], f32)
            ICNT = cpool.tile([G, 1], f32)
            POOLM = cpool.tile([128, NTILE, G], bf)
            X1RES = cpool.tile([128, NTILE, 640], bf)
            ALD1 = cpool.tile([128, NTILE, 16], bf)   # layers 1 and 3
            ALD2 = cpool.tile([128, NTILE, 16], bf)   # layer 2
            XT = cpool.tile([128, 5, SHARD], bf)      # x_l^T for next node phase
            PART = cpool.tile([128, NTILE, 656], f32)  # edge aggregation partials

            # ---- DRAM scratch / tables / collectives ---------------------
            T1sA = dpool.tile([HSH, ROW], bf)
            T1sB = dpool.tile([HSH, ROW], bf)
            T1A = dpool.tile([N_CORES * HSH, ROW], bf, addr_space="Shared")
            T1B = dpool.tile([N_CORES * HSH, ROW], bf, addr_space="Shared")
            T2sA = dpool.tile([HSH, ROW], bf)
            T2sB = dpool.tile([HSH, ROW], bf)
            T2A = dpool.tile([N_CORES * HSH, ROW], bf, addr_space="Shared")
            T2B = dpool.tile([N_CORES * HSH, ROW], bf, addr_space="Shared")
            T3sA = dpool.tile([HSH, ROW3], bf)
            T3sB = dpool.tile([HSH, ROW3], bf)
            T3A = dpool.tile([N_CORES * HSH, ROW3], bf, addr_space="Shared")
            T3B = dpool.tile([N_CORES * HSH, ROW3], bf, addr_space="Shared")
            XS1 = dpool.tile([SHARD, 640], bf)
            XS2 = dpool.tile([SHARD, 640], bf)
            PIN = dpool.tile([G, 16], f32)
            POUT = dpool.tile([G, 16], f32, addr_space="Shared")

            TS = {1: (T1sA, T1sB), 2: (T2sA, T2sB), 3: (T3sA, T3sB)}
            TG = {1: (T1A, T1B), 2: (T2A, T2B), 3: (T3A, T3B)}

            def ag(lyr, half):
                nc.gpsimd.collective_compute(
                    "AllGather", bass.mybir.AluOpType.bypass,
                    replica_groups=RG,
                    ins=[TS[lyr][half].opt()], outs=[TG[lyr][half].opt()])

            def node_tile(lyr, t, lhsT_ap, ald):
                """One 128-node projection tile for layer lyr."""
                We = {1: W1e, 2: W2e, 3: W3e}[lyr]
                hw, hs, rw = HW[lyr], H[lyr], RW[lyr]
                nw = hw + 2 * hs
                nin = We.shape[1]
                ps = psA.tile([128, 768], f32, tag="mm", name="ps")
                r0 = min(512, nw)
                for c in range(nin):
                    nc.tensor.matmul(ps[:, 0:r0], lhsT_ap[:, c, :], We[:, c, 0:r0],
                                     start=(c == 0), stop=(c == nin - 1))
                if nw > 512:
                    for c in range(nin):
                        nc.tensor.matmul(ps[:, 512:nw], lhsT_ap[:, c, :],
                                         We[:, c, 512:nw],
                                         start=(c == 0), stop=(c == nin - 1))
                # staged table row: [h | al_s f32 (2 bf16 slots each) | pad]
                half, r = (0, t) if t < NTILE // 2 else (1, t - NTILE // 2)
                stage_dst = TS[lyr][half][r * 128:(r + 1) * 128, :]
                row = spool.tile([128, ROW], bf, tag="rowstage")
                nc.scalar.activation(row[:, 0:hw], ps[:, 0:hw], AT.Copy)
                nc.vector.tensor_copy(row[:, hw:hw + 2 * hs].bitcast(f32),
                                      ps[:, hw:hw + hs])
                nc.sync.dma_start(stage_dst, row[:, 0:rw])
                # al_d hi/lo resident
                nc.vector.tensor_copy(ald[:, t, 0:hs], ps[:, hw + hs:hw + 2 * hs])
                hi_w = spool.tile([128, 8], f32, tag="histage")
                nc.vector.tensor_copy(hi_w[:, 0:hs], ald[:, t, 0:hs])
                lo2 = spool.tile([128, 8], f32, tag="lostage2")
                nc.vector.tensor_tensor(lo2[:, 0:hs], ps[:, hw + hs:hw + 2 * hs],
                                        hi_w[:, 0:hs], OP.subtract)
                nc.vector.tensor_copy(ald[:, t, hs:2 * hs], lo2[:, 0:hs])

            def edge_phase(lyr, out_cb):
                """Class-A units (all tiles) first, then class-B: the B
                gathers wait on AllGather-B while A compute proceeds.
                Software-pipelined: gathers lead compute by 2 units."""
                hh = H[lyr]
                hw = HW[lyr]
                rw = RW[lyr]
                dvh = DVH[lyr]
                w = hw + hh
                ald = ALD2 if lyr == 2 else ALD1
                gp = gpool3 if lyr == 3 else gpool
                units = []
                for cl in range(2):
                    for t in range(NTILE):
                        n = nch[cl][t]
                        c0 = 0
                        while c0 < n:
                            npc = min(MAXH, n - c0)
                            units.append((t, cl, c0, npc, n))
                            c0 += npc
                nU = len(units)
                G_of = {}
                pso_of = {}

                def gchunk(t, cl, c0):
                    return (totcA if cl else 0) + cbase[cl][t] + c0

                def emit_gather(i):
                    t, cl, c0, npc, n = units[i]
                    Gt = gp.tile([128, MAXH, rw], bf, tag="G", name="Gt")
                    gc = gchunk(t, cl, c0)
                    nidx = npc * 128
                    nc.gpsimd.dma_gather(
                        Gt[:, 0:npc, :], TG[lyr][cl][:],
                        IDX[:, gc * 8:(gc + npc) * 8],
                        num_idxs=nidx, num_idxs_reg=nidx, elem_size=rw,
                        single_packet=True, queue_num=i % 2)
                    G_of[i] = Gt

                def emit_pre(i):
                    """al_d expansion matmul + z = al_s + al_d, alpha=exp(lrelu)."""
                    t, cl, c0, npc, n = units[i]
                    Gt = G_of[i]
                    pz = psZ.tile([128, MAXH, 16], f32, tag="z", name="pz")
                    gc = gchunk(t, cl, c0)
                    for c in range(npc):
                        nc.tensor.matmul(pz[:, c, 0:2 * hh], ST[:, gc + c, :],
                                         ald[:, t, 0:2 * hh], start=True, stop=True)
                    als = Gt[:, 0:npc, hw:hw + 2 * hh].bitcast(f32)
                    zw = spool.tile([128, MAXH, 8], f32, tag="zw")
                    nc.vector.tensor_tensor(zw[:, 0:npc, 0:hh], als,
                                            pz[:, 0:npc, 0:hh], OP.add)
                    nc.vector.tensor_tensor(zw[:, 0:npc, 0:hh], zw[:, 0:npc, 0:hh],
                                            pz[:, 0:npc, hh:2 * hh], OP.add)
                    nc.vector.scalar_tensor_tensor(zw[:, 0:npc, 0:hh],
                                                   zw[:, 0:npc, 0:hh], NEG_SLOPE,
                                                   zw[:, 0:npc, 0:hh], OP.mult, OP.max)
                    nc.scalar.activation(Gt[:, 0:npc, hw:hw + hh],
                                         zw[:, 0:npc, 0:hh], AT.Exp)

                def emit_compute(i):
                    t, cl, c0, npc, n = units[i]
                    Gt = G_of.pop(i)
                    gc = gchunk(t, cl, c0)
                    if (t, cl) not in pso_of:
                        pso_of[(t, cl)] = psA.tile([128, 768], f32, tag="mm",
                                                   name="pso")
                    pso = pso_of[(t, cl)]
                    # messages *= alpha (broadcast AP on DVE)
                    g4 = Gt[:, 0:npc, 0:hw].rearrange(
                        "p c (h x) -> p c h x", h=hh)
                    w4 = Gt[:, 0:npc, hw:hw + hh].unsqueeze(-1).broadcast_to(
                        [128, npc, hh, HID])
                    nc.vector.tensor_tensor(g4, g4, w4, OP.mult)
                    # aggregate into PSUM
                    w1 = min(512, w)
                    first = (c0 == 0)
                    last = (c0 + npc == n)
                    for c in range(npc):
                        st = (first and c == 0)
                        sp = last and (c == npc - 1)
                        nc.tensor.matmul(pso[:, 0:w1], S[:, gc + c, :],
                                         Gt[:, c, 0:w1], start=st, stop=sp,
                                         skip_group_check=True)
                        if w > 512:
                            nc.tensor.matmul(pso[:, 512:w],
                                             S[:, gc + c, :],
                                             Gt[:, c, 512:w], start=st, stop=sp,
                                             skip_group_check=True)
                    if last:
                        pso = pso_of.pop((t, cl))
                        if cl == 0:
                            nc.vector.tensor_copy(PART[:, t, 0:w], pso[:, 0:w])
                        else:
                            # free the PSUM buffer now; everything else is
                            # deferred off the close's critical path
                            nc.vector.tensor_tensor(PART[:, t, 0:w],
                                                    PART[:, t, 0:w],
                                                    pso[:, 0:w], OP.add)

                            def finish(t=t):
                                # normalize: out = num / (den + eps)
                                rec = spool.tile([128, 8], f32, tag="rec")
                                nc.vector.tensor_scalar(rec[:, 0:hh],
                                                        PART[:, t, hw:hw + hh],
                                                        1e-16, None, OP.add,
                                                        OP.bypass)
                                nc.vector.reciprocal(rec[:, 0:hh], rec[:, 0:hh])
                                xt = spool.tile([128, 640], bf, tag="xt")
                                o4 = PART[:, t, 0:hw].rearrange(
                                    "p (h x) -> p h x", h=hh)
                                r4 = rec[:, 0:hh].unsqueeze(-1).broadcast_to(
                                    [128, hh, HID])
                                nc.vector.tensor_tensor(
                                    xt[:, 0:hw].rearrange("p (h x) -> p h x",
                                                          h=hh),
                                    o4, r4, OP.mult)
                                work = out_cb(t, xt)
                                if work is not None:
                                    deferred.append((cur_step[0] + 2, work))
                            deferred.append((cur_step[0] + 2, finish))

                # next-layer projections are emitted 2 units late so their
                # deps (transposed x) are ready when PE reaches them
                deferred = []
                cur_step = [0]
                for i in range(nU + 4):
                    cur_step[0] = i
                    while deferred and deferred[0][0] <= i:
                        deferred.pop(0)[1]()
                    if i < nU:
                        emit_gather(i)
                    if 0 <= i - 1 < nU:
                        emit_pre(i - 1)
                    if 0 <= i - 2 < nU:
                        emit_compute(i - 2)
                while deferred:
                    deferred.pop(0)[1]()

            def stage_next(lyr, t, xs, x_tile):
                """Write x_{l} tile to DRAM, transpose back per-tile, project
                next layer's node tile, trigger AllGathers at tiles 4/9."""
                nc.sync.dma_start(xs[t * 128:(t + 1) * 128, :], x_tile[:])
                for c in range(5):
                    nc.sync.dma_start(
                        XT[:, c, t * 128:(t + 1) * 128],
                        xs[t * 128:(t + 1) * 128, c * 128:(c + 1) * 128],
                        transpose=True)

                def project(lyr=lyr, t=t):
                    node_tile(lyr, t, XT[:, :, t * 128:(t + 1) * 128],
                              ALD2 if lyr == 2 else ALD1)
                    if t == 6:  # tiles 0-4 staged well before; cheap trigger wait
                        ag(lyr, 0)
                    elif t == NTILE - 1:
                        ag(lyr, 1)
                return project

            # PE warm-up: dense dummy matmuls flip the HAM clock gate to 8/8
            # while the first input DMAs land (values are never read)
            wsrc = spool.tile([128, 512], bf, tag="warm")
            nc.vector.memset(wsrc[:], 0.0)
            wps = psA.tile([128, 512], f32, tag="mm", name="wps")
            for _ in range(12):
                nc.tensor.matmul(wps[:], wsrc[:, 0:128], wsrc[:],
                                 start=True, stop=True, skip_group_check=True)

            # =================== LAYER 1 node phase =======================
            nc.sync.dma_start(W1e[:], d_W1e[:].rearrange("(c p) w -> p c w", p=128))
            for t in range(NTILE):
                lx = mpool.tile([128, 2, 128], bf, tag="lx")
                nc.sync.dma_start(lx[:], d_xtl[:].rearrange("(c p) n -> p c n", p=128)
                                  [:, :, t * 128:(t + 1) * 128])
                node_tile(1, t, lx[:], ALD1)
                if t == NTILE // 2 - 1:
                    ag(1, 0)
                elif t == NTILE - 1:
                    ag(1, 1)

            # remaining resident constants (overlap AllGather-1)
            nc.sync.dma_start(IDX[:], d_idx[:])
            nc.sync.dma_start(S[:], d_S[:])
            nc.sync.dma_start(ST[:], d_ST[:])
            nc.sync.dma_start(W2e[:], d_W2e[:].rearrange("(c p) w -> p c w", p=128))
            nc.sync.dma_start(W3e[:], d_W3e[:].rearrange("(c p) w -> p c w", p=128))
            nc.sync.dma_start(B1[:], d_b1[:])
            nc.sync.dma_start(B2[:], d_b2[:])
            nc.sync.dma_start(B3[:], d_b3[:])
            nc.sync.dma_start(WC[:], d_Wc[:])
            nc.sync.dma_start(BC[:], d_bcr[:])
            nc.sync.dma_start(ICNT[:], d_icnt[:])
            nc.sync.dma_start(POOLM[:], d_pool[:])

            # =================== LAYER 1 edge phase =======================
            def l1_out(t, xt):
                u = spool.tile([128, 640], bf, tag="u")
                nc.vector.tensor_tensor(u[:], xt[:], B1[:], OP.add)
                m = spool.tile([128, 640], bf, tag="m")
                nc.vector.tensor_scalar(m[:], u[:], 0.0, None, OP.min, OP.bypass)
                e = spool.tile([128, 640], bf, tag="e")
                nc.scalar.activation(e[:], m[:], AT.Exp)
                nc.vector.scalar_tensor_tensor(X1RES[:, t, :], u[:], 0.0, e[:],
                                               OP.max, OP.add)
                nc.vector.tensor_scalar(X1RES[:, t, :], X1RES[:, t, :], -1.0, None,
                                        OP.add, OP.bypass)
                return stage_next(2, t, XS1, X1RES[:, t, :])
            edge_phase(1, l1_out)

            # =================== LAYER 2 edge phase =======================
            def l2_out(t, xt):
                u = spool.tile([128, 640], bf, tag="u")
                nc.vector.tensor_tensor(u[:], xt[:], B2[:], OP.add)
                nc.vector.tensor_tensor(u[:], u[:], X1RES[:, t, :], OP.add)
                m = spool.tile([128, 640], bf, tag="m")
                nc.vector.tensor_scalar(m[:], u[:], 0.0, None, OP.min, OP.bypass)
                e = spool.tile([128, 640], bf, tag="e")
                nc.scalar.activation(e[:], m[:], AT.Exp)
                x2 = spool.tile([128, 640], bf, tag="x2")
                nc.vector.scalar_tensor_tensor(x2[:], u[:], 0.0, e[:], OP.max, OP.add)
                nc.vector.tensor_scalar(x2[:], x2[:], -1.0, None, OP.add, OP.bypass)
                return stage_next(3, t, XS2, x2[:])
            edge_phase(2, l2_out)

            # =================== LAYER 3 edge phase + pool ================
            ppool = psP.tile([128, G], f32)

            def l3_out(t, xt):
                s = spool.tile([128, 128], f32, tag="s3")
                nc.vector.tensor_tensor(s[:], xt[:, 0:128], xt[:, 128:256], OP.add)
                nc.vector.tensor_tensor(s[:], s[:], xt[:, 256:384], OP.add)
                x3 = spool.tile([128, 128], bf, tag="x3")
                nc.vector.scalar_tensor_tensor(x3[:], s[:], 1.0 / 3.0, B3[:],
                                               OP.mult, OP.add)
                nc.tensor.matmul(ppool[:], x3[:], POOLM[:, t, :],
                                 start=(t == 0), stop=(t == NTILE - 1))
            edge_phase(3, l3_out)

            # classifier partial before AllReduce: [64, 16] = pooled^T @ Wc
            pp = spool.tile([128, G], bf, tag="pp")
            nc.vector.tensor_copy(pp[:], ppool[:])
            psl = psZ.tile([G, 16], f32, tag="z")
            nc.tensor.matmul(psl[:], pp[:], WC[:], start=True, stop=True)
            lgp = spool.tile([G, 16], f32, tag="lgp")
            nc.vector.tensor_copy(lgp[:], psl[:])
            nc.sync.dma_start(PIN[:], lgp[:])
            nc.gpsimd.collective_compute(
                "AllReduce", bass.mybir.AluOpType.add,
                replica_groups=RG,
                ins=[PIN.opt()], outs=[POUT.opt()])
            arf = spool.tile([G, 16], f32, tag="arf")
            nc.sync.dma_start(arf[:], POUT[:])
            lg = spool.tile([G, NC_CLS], f32, tag="lg2")
            nc.vector.tensor_scalar(lg[:], arf[:, 0:NC_CLS], ICNT[:], None,
                                    OP.mult, OP.bypass)
            nc.vector.tensor_tensor(lg[:], lg[:], BC[:], OP.add)
            # log_softmax over free dim (10)
            mx = spool.tile([G, 1], f32, tag="mx")
            nc.vector.tensor_reduce(mx[:], lg[:], mybir.AxisListType.X, OP.max)
            sh = spool.tile([G, NC_CLS], f32, tag="sh")
            nc.vector.tensor_scalar(sh[:], lg[:], mx[:], None, OP.subtract, OP.bypass)
            ex = spool.tile([G, NC_CLS], f32, tag="ex")
            nc.scalar.activation(ex[:], sh[:], AT.Exp)
            sm = spool.tile([G, 1], f32, tag="sm")
            nc.vector.tensor_reduce(sm[:], ex[:], mybir.AxisListType.X, OP.add)
            nc.scalar.activation(sm[:], sm[:], AT.Ln)
            lp = spool.tile([G, NC_CLS], f32, tag="lp")
            nc.vector.tensor_scalar(lp[:], sh[:], sm[:], None, OP.subtract, OP.bypass)
            nc.sync.dma_start(d_out[0], lg[:])
            nc.sync.dma_start(d_out[1], lp[:])

    nc.compile()
    return nc


_CACHE = {}


def kernel(**inputs):
    from concourse.bass_utils import run_bass_kernel_spmd

    x = np.asarray(inputs["x"], np.float32)
    edge_index = np.asarray(inputs["edge_index"], np.int64)
    batch = np.asarray(inputs["batch"], np.int64)
    weights = [np.asarray(inputs[k], np.float32) for k in
               ["W1", "a1s", "a1d", "b1", "W2", "a2s", "a2d", "b2",
                "W3", "a3s", "a3d", "b3", "Wc", "bc"]]

    per_core, nch, totc = _build_host_data(x, edge_index, batch, weights)

    key = (tuple(nch[0]), tuple(nch[1]))
    if key not in _CACHE:
        _CACHE[key] = _build_bass(nch, totc)
    nc = _CACHE[key]

    in_maps = [per_core[k] for k in range(N_CORES)]
    last_err = None
    out = None
    for attempt in range(10):
        try:
            res = run_bass_kernel_spmd(nc, in_maps, core_ids=list(range(N_CORES)))
            out = res.results[0]["out"]
            if np.all(np.isfinite(out)):
                return (np.asarray(out[0], np.float32),
                        np.asarray(out[1], np.float32))
        except Exception as e:  # transient NRT/device failures: retry
            last_err = e
            import time
            time.sleep(min(2 + 2 * attempt, 10))
    if out is not None:
        return np.asarray(out[0], np.float32), np.asarray(out[1], np.float32)
    raise last_err


if __name__ == "__main__":
    sys.path.insert(0, '/root/problem')
    import reference
    ins = {k: np.asarray(v) for k, v in reference.setup_inputs().items()}
    got = kernel(**ins)
    exp = reference.reference(**ins)
    for g_, e_ in zip(got, exp):
        e_ = np.asarray(e_)
        err = np.abs(g_ - e_).max() / (np.abs(e_).max() + 1e-9)
        print("rel err:", err)


# revision 50
# speedup vs baseline: 1.0810x; 1.0810x over previous
"""GAT (3-layer, multi-head) forward on 8 Trainium2 NeuronCores.

Strategy (graph/data parallel, per sharding hint):
- Nodes sharded in contiguous 1280-node blocks (10 tiles of 128) per core;
  edges partitioned by destination, then split into two classes by the
  source node's position within its owner core's shard (first/second 640
  rows), sorted by src within each (dst-tile, class) for HBM locality.
- Per layer: node-phase projection (PE matmul), table rows
  [h bf16 | al_src f32 | pad] staged to HBM in two halves; two AllGathers
  (A = every core's first 640 rows, B = second 640) so the class-A edge
  compute overlaps the AllGather-B wire time. The next layer's node
  projection and all normalize/ELU/staging chains are deferred two
  pipeline units past each aggregation close so they never head-block
  the PE/DVE queues.
- Edge phase per (dst-tile, class): dma_gather of source rows (the only
  per-edge data movement, alternating between 2 SWDGE queues), al_dst
  expanded edge-wise via one-hot^T matmul on PE (one-hots stored fp8,
  exact), attention weights exp(leakyrelu(z)) on ACT, applied to messages
  with one broadcast-AP DVE multiply, aggregation + softmax denominators
  via one-hot matmul into PSUM, class partials combined in SBUF, then
  normalize/bias/ELU/residual. A startup dummy-matmul burst warms the
  PE clock gate under the first DMAs.
- Graph mean-pool via one-hot matmul accumulated over tiles, classifier
  matmul applied to the per-core partial, tiny [64,16] AllReduce, then
  log_softmax computed redundantly on every core.
"""
import sys

sys.path.insert(0, '/opt/trn_rl_repo')

import numpy as np

N_CORES = 8
N, E, F_IN, HID, H1, H3, NC_CLS, G = 10000, 160000, 256, 128, 5, 3, 10, 64
NEG_SLOPE = 0.2
SHARD = 1280            # nodes per core (10 tiles of 128); core 7 partially padded
NTILE = 10              # dst tiles per core
HSH = 640               # half-shard rows per AllGather piece
NPAD = SHARD * N_CORES  # 10240 padded node count
ROW = 768               # L1/L2 table row elems (bf16): [h 640 | al_s hi 5 | lo 5 | pad]
ROW3 = 512              # L3 table row elems: [h 384 | al hi 3 | al lo 3 | pad]
MAXH = 5                # max chunks per gather unit


def _blockdiag_a(a):
    Hh, C = a.shape
    out = np.zeros((Hh * C, Hh), np.float32)
    for h in range(Hh):
        out[h * C:(h + 1) * C, h] = a[h]
    return out


def _build_host_data(x, edge_index, batch, weights):
    """All index preprocessing + per-core constant inputs."""
    import ml_dtypes
    bf16 = ml_dtypes.bfloat16
    f8 = ml_dtypes.float8_e4m3

    src = np.concatenate([edge_index[0], np.arange(N)]).astype(np.int64)
    dst = np.concatenate([edge_index[1], np.arange(N)]).astype(np.int64)

    core_of = dst // SHARD
    tile_of = (dst % SHARD) // 128
    cls_of = ((src % SHARD) >= HSH).astype(np.int64)  # 0=A, 1=B

    counts = np.zeros((N_CORES, NTILE, 2), np.int64)
    for k in range(N_CORES):
        for t in range(NTILE):
            for cl in range(2):
                counts[k, t, cl] = np.count_nonzero(
                    (core_of == k) & (tile_of == t) & (cls_of == cl))
    nch = [[int(np.ceil(counts[:, t, cl].max() / 128)) for t in range(NTILE)]
           for cl in range(2)]
    assert all(n > 0 for n in nch[0]) and all(n > 0 for n in nch[1])
    totcA, totcB = sum(nch[0]), sum(nch[1])
    totc = totcA + totcB

    # remapped gather index: class A -> 640*k + r, class B -> 640*k + r - 640
    remap = HSH * (src // SHARD) + (src % SHARD) - cls_of * HSH

    per_core = []
    for k in range(N_CORES):
        srcs = np.zeros(totc * 128, np.int16)
        S = np.zeros((128, totc, 128), np.float32)
        base = 0
        nreal = {}
        for cl in range(2):
            for t in range(NTILE):
                m = (core_of == k) & (tile_of == t) & (cls_of == cl)
                mi = np.nonzero(m)[0]
                mi = mi[np.argsort(src[mi], kind='stable')]  # src-sorted: HBM locality
                n = len(mi)
                nreal[(cl, t)] = n
                cap = nch[cl][t] * 128
                sl = np.zeros(cap, np.int16)  # pads gather row 0 (S col is zero)
                sl[:n] = remap[mi].astype(np.int16)
                srcs[base * 128:(base + nch[cl][t]) * 128] = sl
                dloc = (dst[mi] % SHARD) % 128
                e = np.arange(n)
                S[e % 128, base + e // 128, dloc] = 1.0
                base += nch[cl][t]
        ST = np.ascontiguousarray(S.transpose(2, 1, 0))  # [128 d, totc, 128 e]

        def wrap_idx(flat):
            n_ = len(flat)
            cols = n_ // 16
            t_ = np.zeros((128, cols), np.int16)
            v = flat.reshape(cols, 16).T
            for g_ in range(8):
                t_[g_ * 16:(g_ + 1) * 16, :] = v
            return t_

        pool = np.zeros((128, NTILE, G), np.float32)
        for t in range(NTILE):
            gn = SHARD * k + 128 * t + np.arange(128)
            ok = gn < N
            pool[ok, t, batch[gn[ok]]] = 1.0

        per_core.append(dict(
            src_idx=wrap_idx(srcs),
            S=S.astype(f8), ST=ST.astype(f8),
            pool=pool.astype(bf16),
        ))

    cnt = np.bincount(batch, minlength=G).astype(np.float32)
    inv_cnt = (1.0 / np.maximum(cnt, 1.0)).astype(np.float32)

    W1, a1s, a1d, b1, W2, a2s, a2d, b2, W3, a3s, a3d, b3, Wc, bc = weights

    def ext(W, as_, ad_, width):
        A_s = W @ _blockdiag_a(as_)
        A_d = W @ _blockdiag_a(ad_)
        out = np.zeros((W.shape[0], width), np.float32)
        out[:, :W.shape[1]] = W
        out[:, W.shape[1]:W.shape[1] + as_.shape[0]] = A_s
        out[:, W.shape[1] + as_.shape[0]:W.shape[1] + 2 * as_.shape[0]] = A_d
        return out

    W1e = ext(W1, a1s, a1d, 656).astype(bf16)   # cols: 640 h, 640:645 als, 645:650 ald
    W2e = ext(W2, a2s, a2d, 656).astype(bf16)
    W3e = ext(W3, a3s, a3d, 392).astype(bf16)   # 384 h, 384:387 als, 387:390 ald

    xt_full = np.zeros((F_IN, NPAD), np.float32)
    xt_full[:, :N] = x.T
    xt_full = xt_full.astype(bf16)

    consts = dict(
        W1e=W1e, W2e=W2e, W3e=W3e,
        b1r=np.tile(b1[None, :], (128, 1)).astype(np.float32),
        b2r=np.tile(b2[None, :], (128, 1)).astype(np.float32),
        b3r=np.tile(b3[None, :], (128, 1)).astype(np.float32),
        Wc=np.concatenate([Wc, np.zeros((HID, 6), np.float32)], 1).astype(bf16),
        bcr=np.tile(bc[None, :], (G, 1)).astype(np.float32),
        inv_cnt=inv_cnt.reshape(G, 1),
    )
    for k in range(N_CORES):
        xt_loc = np.asarray(xt_full[:, SHARD * k:SHARD * (k + 1)])
        per_core[k]['xt_loc'] = np.ascontiguousarray(xt_loc)
        per_core[k].update(consts)
    return per_core, nch, totc


def _build_bass(nch, totc):
    import concourse.bass as bass
    import concourse.mybir as mybir
    import concourse.tile as tile
    from concourse import bacc

    bf = mybir.dt.bfloat16
    f8 = mybir.dt.float8e4
    f32 = mybir.dt.float32
    AT = mybir.ActivationFunctionType
    OP = mybir.AluOpType
    RG = [list(range(N_CORES))]

    totcA = sum(nch[0])

    nc = bacc.Bacc("TRN2", target_bir_lowering=False, debug=False,
                   num_devices=N_CORES, num_swdge_queues=2)

    # ---- inputs -----------------------------------------------------------
    d_xtl = nc.dram_tensor("xt_loc", [F_IN, SHARD], bf, kind="ExternalInput")
    d_W1e = nc.dram_tensor("W1e", [F_IN, 656], bf, kind="ExternalInput")
    d_W2e = nc.dram_tensor("W2e", [640, 656], bf, kind="ExternalInput")
    d_W3e = nc.dram_tensor("W3e", [640, 392], bf, kind="ExternalInput")
    d_b1 = nc.dram_tensor("b1r", [128, 640], f32, kind="ExternalInput")
    d_b2 = nc.dram_tensor("b2r", [128, 640], f32, kind="ExternalInput")
    d_b3 = nc.dram_tensor("b3r", [128, 128], f32, kind="ExternalInput")
    d_Wc = nc.dram_tensor("Wc", [HID, 16], bf, kind="ExternalInput")
    d_bcr = nc.dram_tensor("bcr", [G, NC_CLS], f32, kind="ExternalInput")
    d_icnt = nc.dram_tensor("inv_cnt", [G, 1], f32, kind="ExternalInput")
    d_S = nc.dram_tensor("S", [128, totc, 128], f8, kind="ExternalInput")
    d_ST = nc.dram_tensor("ST", [128, totc, 128], f8, kind="ExternalInput")
    d_idx = nc.dram_tensor("src_idx", [128, totc * 8], mybir.dt.int16, kind="ExternalInput")
    d_pool = nc.dram_tensor("pool", [128, NTILE, G], bf, kind="ExternalInput")
    d_out = nc.dram_tensor("out", [2, G, NC_CLS], f32, kind="ExternalOutput")

    H = {1: H1, 2: H1, 3: H3}
    HW = {1: 640, 2: 640, 3: 384}       # h width per layer
    RW = {1: ROW, 2: ROW, 3: ROW3}      # table row width per layer
    DVH = {1: 3, 2: 3, 3: 2}            # heads multiplied on DVE (rest on ACT)
    cbase = [[0], [0]]
    for cl in range(2):
        for t in range(NTILE):
            cbase[cl].append(cbase[cl][-1] + nch[cl][t])

    with tile.TileContext(nc) as tc:
        with (
            tc.tile_pool(name="const", bufs=1) as cpool,
            tc.tile_pool(name="mm", bufs=4) as mpool,
            tc.tile_pool(name="gath", bufs=5) as gpool,
            tc.tile_pool(name="gath3", bufs=4) as gpool3,
            tc.tile_pool(name="small", bufs=2) as spool,
            tc.tile_pool(name="psA", bufs=3, space="PSUM") as psA,
            tc.tile_pool(name="psZ", bufs=1, space="PSUM") as psZ,
            tc.tile_pool(name="psP", bufs=1, space="PSUM") as psP,
            tc.tile_pool(name="dram", bufs=1, space="DRAM") as dpool,
        ):
            # ---- resident constants --------------------------------------
            S = cpool.tile([128, totc, 128], f8)
            ST = cpool.tile([128, totc, 128], f8)
            IDX = cpool.tile([128, totc * 8], mybir.dt.int16)
            W1e = cpool.tile([128, 2, 656], bf)
            W2e = cpool.tile([128, 5, 656], bf)
            W3e = cpool.tile([128, 5, 392], bf)
            B1 = cpool.tile([128, 640], f32)
            B2 = cpool.tile([128, 640], f32)
            B3 = cpool.tile([128, 128], f32)
            WC = cpool.tile([128, 16], bf)
            BC = cpool.tile([G, NC_CLS], f32)
            ICNT = cpool.tile([G, 1], f32)
            POOLM = cpool.tile([128, NTILE, G], bf)
            X1RES = cpool.tile([128, NTILE, 640], bf)
            ALD1 = cpool.tile([128, NTILE, 16], bf)   # layers 1 and 3
            ALD2 = cpool.tile([128, NTILE, 16], bf)   # layer 2
            XT = cpool.tile([128, 5, SHARD], bf)      # x_l^T for next node phase
            PART = cpool.tile([128, NTILE, 656], f32)  # edge aggregation partials

            # ---- DRAM scratch / tables / collectives ---------------------
            T1sA = dpool.tile([HSH, ROW], bf)
            T1sB = dpool.tile([HSH, ROW], bf)
            T1A = dpool.tile([N_CORES * HSH, ROW], bf, addr_space="Shared")
            T1B = dpool.tile([N_CORES * HSH, ROW], bf, addr_space="Shared")
            T2sA = dpool.tile([HSH, ROW], bf)
            T2sB = dpool.tile([HSH, ROW], bf)
            T2A = dpool.tile([N_CORES * HSH, ROW], bf, addr_space="Shared")
            T2B = dpool.tile([N_CORES * HSH, ROW], bf, addr_space="Shared")
            T3sA = dpool.tile([HSH, ROW3], bf)
            T3sB = dpool.tile([HSH, ROW3], bf)
            T3A = dpool.tile([N_CORES * HSH, ROW3], bf, addr_space="Shared")
            T3B = dpool.tile([N_CORES * HSH, ROW3], bf, addr_space="Shared")
            XS1 = dpool.tile([SHARD, 640], bf)
            XS2 = dpool.tile([SHARD, 640], bf)
            PIN = dpool.tile([G, 16], f32)
            POUT = dpool.tile([G, 16], f32, addr_space="Shared")

            TS = {1: (T1sA, T1sB), 2: (T2sA, T2sB), 3: (T3sA, T3sB)}
            TG = {1: (T1A, T1B), 2: (T2A, T2B), 3: (T3A, T3B)}

            def ag(lyr, half):
                nc.gpsimd.collective_compute(
                    "AllGather", bass.mybir.AluOpType.bypass,
                    replica_groups=RG,
                    ins=[TS[lyr][half].opt()], outs=[TG[lyr][half].opt()])

            def node_tile(lyr, t, lhsT_ap, ald):
                """One 128-node projection tile for layer lyr."""
                We = {1: W1e, 2: W2e, 3: W3e}[lyr]
                hw, hs, rw = HW[lyr], H[lyr], RW[lyr]
                nw = hw + 2 * hs
                nin = We.shape[1]
                ps = psA.tile([128, 768], f32, tag="mm", name="ps")
                r0 = min(512, nw)
                for c in range(nin):
                    nc.tensor.matmul(ps[:, 0:r0], lhsT_ap[:, c, :], We[:, c, 0:r0],
                                     start=(c == 0), stop=(c == nin - 1))
                if nw > 512:
                    for c in range(nin):
                        nc.tensor.matmul(ps[:, 512:nw], lhsT_ap[:, c, :],
                                         We[:, c, 512:nw],
                                         start=(c == 0), stop=(c == nin - 1))
                # staged table row: [h | al_s f32 (2 bf16 slots each) | pad]
                half, r = (0, t) if t < NTILE // 2 else (1, t - NTILE // 2)
                stage_dst = TS[lyr][half][r * 128:(r + 1) * 128, :]
                row = spool.tile([128, ROW], bf, tag="rowstage")
                nc.scalar.activation(row[:, 0:hw], ps[:, 0:hw], AT.Copy)
                nc.vector.tensor_copy(row[:, hw:hw + 2 * hs].bitcast(f32),
                                      ps[:, hw:hw + hs])
                nc.sync.dma_start(stage_dst, row[:, 0:rw])
                # al_d hi/lo resident
                nc.vector.tensor_copy(ald[:, t, 0:hs], ps[:, hw + hs:hw + 2 * hs])
                hi_w = spool.tile([128, 8], f32, tag="histage")
                nc.vector.tensor_copy(hi_w[:, 0:hs], ald[:, t, 0:hs])
                lo2 = spool.tile([128, 8], f32, tag="lostage2")
                nc.vector.tensor_tensor(lo2[:, 0:hs], ps[:, hw + hs:hw + 2 * hs],
                                        hi_w[:, 0:hs], OP.subtract)
                nc.vector.tensor_copy(ald[:, t, hs:2 * hs], lo2[:, 0:hs])

            def edge_phase(lyr, out_cb):
                """Class-A units (all tiles) first, then class-B: the B
                gathers wait on AllGather-B while A compute proceeds.
                Software-pipelined: gathers lead compute by 2 units."""
                hh = H[lyr]
                hw = HW[lyr]
                rw = RW[lyr]
                dvh = DVH[lyr]
                w = hw + hh
                ald = ALD2 if lyr == 2 else ALD1
                gp = gpool3 if lyr == 3 else gpool
                units = []
                for cl in range(2):
                    for t in range(NTILE):
                        n = nch[cl][t]
                        c0 = 0
                        while c0 < n:
                            npc = min(MAXH, n - c0)
                            units.append((t, cl, c0, npc, n))
                            c0 += npc
                nU = len(units)
                G_of = {}
                pz_of = {}
                pso_of = {}

                def gchunk(t, cl, c0):
                    return (totcA if cl else 0) + cbase[cl][t] + c0

                def emit_gather(i):
                    t, cl, c0, npc, n = units[i]
                    Gt = gp.tile([128, MAXH, rw], bf, tag="G", name="Gt")
                    gc = gchunk(t, cl, c0)
                    nidx = npc * 128
                    nc.gpsimd.dma_gather(
                        Gt[:, 0:npc, :], TG[lyr][cl][:],
                        IDX[:, gc * 8:(gc + npc) * 8],
                        num_idxs=nidx, num_idxs_reg=nidx, elem_size=rw,
                        single_packet=False, queue_num=i % 2)
                    G_of[i] = Gt

                def emit_pre(i):
                    """al_d expansion matmul + z = al_s + al_d, alpha=exp(lrelu)."""
                    t, cl, c0, npc, n = units[i]
                    Gt = G_of[i]
                    pz = psZ.tile([128, MAXH, 16], f32, tag="z", name="pz")
                    gc = gchunk(t, cl, c0)
                    for c in range(npc):
                        nc.tensor.matmul(pz[:, c, 0:2 * hh], ST[:, gc + c, :],
                                         ald[:, t, 0:2 * hh], start=True, stop=True)
                    pz_of[i] = pz
                    als = Gt[:, 0:npc, hw:hw + 2 * hh].bitcast(f32)
                    zw = spool.tile([128, MAXH, 8], f32, tag="zw")
                    nc.vector.tensor_tensor(zw[:, 0:npc, 0:hh], als,
                                            pz[:, 0:npc, 0:hh], OP.add)
                    nc.vector.tensor_tensor(zw[:, 0:npc, 0:hh], zw[:, 0:npc, 0:hh],
                                            pz[:, 0:npc, hh:2 * hh], OP.add)
                    nc.vector.scalar_tensor_tensor(zw[:, 0:npc, 0:hh],
                                                   zw[:, 0:npc, 0:hh], NEG_SLOPE,
                                                   zw[:, 0:npc, 0:hh], OP.mult, OP.max)
                    nc.scalar.activation(Gt[:, 0:npc, hw:hw + hh],
                                         zw[:, 0:npc, 0:hh], AT.Exp)

                def emit_compute(i):
                    t, cl, c0, npc, n = units[i]
                    Gt = G_of.pop(i)
                    gc = gchunk(t, cl, c0)
                    if (t, cl) not in pso_of:
                        pso_of[(t, cl)] = psA.tile([128, 768], f32, tag="mm",
                                                   name="pso")
                    pso = pso_of[(t, cl)]
                    # messages *= alpha (broadcast AP on DVE)
                    g4 = Gt[:, 0:npc, 0:hw].rearrange(
                        "p c (h x) -> p c h x", h=hh)
                    w4 = Gt[:, 0:npc, hw:hw + hh].unsqueeze(-1).broadcast_to(
                        [128, npc, hh, HID])
                    nc.vector.tensor_tensor(g4, g4, w4, OP.mult)
                    # aggregate into PSUM
                    w1 = min(512, w)
                    first = (c0 == 0)
                    last = (c0 + npc == n)
                    for c in range(npc):
                        st = (first and c == 0)
                        sp = last and (c == npc - 1)
                        nc.tensor.matmul(pso[:, 0:w1], S[:, gc + c, :],
                                         Gt[:, c, 0:w1], start=st, stop=sp,
                                         skip_group_check=True)
                        if w > 512:
                            nc.tensor.matmul(pso[:, 512:w],
                                             S[:, gc + c, :],
                                             Gt[:, c, 512:w], start=st, stop=sp,
                                             skip_group_check=True)
                    # filler matmuls into this unit's consumed pz tile: keep
                    # the PE activity monitor from re-throttling the clock to
                    # 4/8 during the wait for the next unit's DVE multiply
                    pzj = pz_of.pop(i)
                    for _ in range(8):
                        nc.tensor.matmul(pzj[:], wsrc[:, 0:128],
                                         wsrc[:, 0:MAXH * 16],
                                         start=True, stop=True,
                                         skip_group_check=True)
                    if last:
                        pso = pso_of.pop((t, cl))
                        if cl == 0:
                            nc.vector.tensor_copy(PART[:, t, 0:w], pso[:, 0:w])
                        else:
                            # free the PSUM buffer now; everything else is
                            # deferred off the close's critical path
                            nc.vector.tensor_tensor(PART[:, t, 0:w],
                                                    PART[:, t, 0:w],
                                                    pso[:, 0:w], OP.add)

                            def finish(t=t):
                                # normalize: out = num / (den + eps)
                                rec = spool.tile([128, 8], f32, tag="rec")
                                nc.vector.tensor_scalar(rec[:, 0:hh],
                                                        PART[:, t, hw:hw + hh],
                                                        1e-16, None, OP.add,
                                                        OP.bypass)
                                nc.vector.reciprocal(rec[:, 0:hh], rec[:, 0:hh])
                                xt = spool.tile([128, 640], bf, tag="xt")
                                o4 = PART[:, t, 0:hw].rearrange(
                                    "p (h x) -> p h x", h=hh)
                                r4 = rec[:, 0:hh].unsqueeze(-1).broadcast_to(
                                    [128, hh, HID])
                                nc.vector.tensor_tensor(
                                    xt[:, 0:hw].rearrange("p (h x) -> p h x",
                                                          h=hh),
                                    o4, r4, OP.mult)
                                work = out_cb(t, xt)
                                if work is not None:
                                    deferred.append((cur_step[0] + 2, work))
                            deferred.append((cur_step[0] + 2, finish))

                # next-layer projections are emitted 2 units late so their
                # deps (transposed x) are ready when PE reaches them
                deferred = []
                cur_step = [0]
                for i in range(nU + 4):
                    cur_step[0] = i
                    while deferred and deferred[0][0] <= i:
                        deferred.pop(0)[1]()
                    if i < nU:
                        emit_gather(i)
                    if 0 <= i - 1 < nU:
                        emit_pre(i - 1)
                    if 0 <= i - 2 < nU:
                        emit_compute(i - 2)
                while deferred:
                    deferred.pop(0)[1]()

            def stage_next(lyr, t, xs, x_tile):
                """Write x_{l} tile to DRAM, transpose back per-tile, project
                next layer's node tile, trigger AllGathers at tiles 4/9."""
                nc.sync.dma_start(xs[t * 128:(t + 1) * 128, :], x_tile[:])
                for c in range(5):
                    nc.sync.dma_start(
                        XT[:, c, t * 128:(t + 1) * 128],
                        xs[t * 128:(t + 1) * 128, c * 128:(c + 1) * 128],
                        transpose=True)

                def project(lyr=lyr, t=t):
                    node_tile(lyr, t, XT[:, :, t * 128:(t + 1) * 128],
                              ALD2 if lyr == 2 else ALD1)
                    if t == 6:  # tiles 0-4 staged well before; cheap trigger wait
                        ag(lyr, 0)
                    elif t == NTILE - 1:
                        ag(lyr, 1)
                return project

            # PE warm-up: dense dummy matmuls flip the HAM clock gate to 8/8
            # while the first input DMAs land (values are never read)
            wsrc = spool.tile([128, 512], bf, tag="warm")
            nc.vector.memset(wsrc[:], 0.0)
            wps = psA.tile([128, 512], f32, tag="mm", name="wps")
            for _ in range(12):
                nc.tensor.matmul(wps[:], wsrc[:, 0:128], wsrc[:],
                                 start=True, stop=True, skip_group_check=True)

            # =================== LAYER 1 node phase =======================
            nc.sync.dma_start(W1e[:], d_W1e[:].rearrange("(c p) w -> p c w", p=128))
            for t in range(NTILE):
                lx = mpool.tile([128, 2, 128], bf, tag="lx")
                nc.sync.dma_start(lx[:], d_xtl[:].rearrange("(c p) n -> p c n", p=128)
                                  [:, :, t * 128:(t + 1) * 128])
                node_tile(1, t, lx[:], ALD1)
                if t == NTILE // 2 - 1:
                    ag(1, 0)
                elif t == NTILE - 1:
                    ag(1, 1)

            # remaining resident constants (overlap AllGather-1)
            nc.sync.dma_start(IDX[:], d_idx[:])
            nc.sync.dma_start(S[:], d_S[:])
            nc.sync.dma_start(ST[:], d_ST[:])
            nc.sync.dma_start(W2e[:], d_W2e[:].rearrange("(c p) w -> p c w", p=128))
            nc.sync.dma_start(W3e[:], d_W3e[:].rearrange("(c p) w -> p c w", p=128))
            nc.sync.dma_start(B1[:], d_b1[:])
            nc.sync.dma_start(B2[:], d_b2[:])
            nc.sync.dma_start(B3[:], d_b3[:])
            nc.sync.dma_start(WC[:], d_Wc[:])
            nc.sync.dma_start(BC[:], d_bcr[:])
            nc.sync.dma_start(ICNT[:], d_icnt[:])
            nc.sync.dma_start(POOLM[:], d_pool[:])

            # =================== LAYER 1 edge phase =======================
            def l1_out(t, xt):
                u = spool.tile([128, 640], bf, tag="u")
                nc.vector.tensor_tensor(u[:], xt[:], B1[:], OP.add)
                m = spool.tile([128, 640], bf, tag="m")
                nc.vector.tensor_scalar(m[:], u[:], 0.0, None, OP.min, OP.bypass)
                e = spool.tile([128, 640], bf, tag="e")
                nc.scalar.activation(e[:], m[:], AT.Exp)
                nc.vector.scalar_tensor_tensor(X1RES[:, t, :], u[:], 0.0, e[:],
                                               OP.max, OP.add)
                nc.vector.tensor_scalar(X1RES[:, t, :], X1RES[:, t, :], -1.0, None,
                                        OP.add, OP.bypass)
                return stage_next(2, t, XS1, X1RES[:, t, :])
            edge_phase(1, l1_out)

            # =================== LAYER 2 edge phase =======================
            def l2_out(t, xt):
                u = spool.tile([128, 640], bf, tag="u")
                nc.vector.tensor_tensor(u[:], xt[:], B2[:], OP.add)
                nc.vector.tensor_tensor(u[:], u[:], X1RES[:, t, :], OP.add)
                m = spool.tile([128, 640], bf, tag="m")
                nc.vector.tensor_scalar(m[:], u[:], 0.0, None, OP.min, OP.bypass)
                e = spool.tile([128, 640], bf, tag="e")
                nc.scalar.activation(e[:], m[:], AT.Exp)
                x2 = spool.tile([128, 640], bf, tag="x2")
                nc.vector.scalar_tensor_tensor(x2[:], u[:], 0.0, e[:], OP.max, OP.add)
                nc.vector.tensor_scalar(x2[:], x2[:], -1.0, None, OP.add, OP.bypass)
                return stage_next(3, t, XS2, x2[:])
            edge_phase(2, l2_out)

            # =================== LAYER 3 edge phase + pool ================
            ppool = psP.tile([128, G], f32)

            def l3_out(t, xt):
                s = spool.tile([128, 128], f32, tag="s3")
                nc.vector.tensor_tensor(s[:], xt[:, 0:128], xt[:, 128:256], OP.add)
                nc.vector.tensor_tensor(s[:], s[:], xt[:, 256:384], OP.add)
                x3 = spool.tile([128, 128], bf, tag="x3")
                nc.vector.scalar_tensor_tensor(x3[:], s[:], 1.0 / 3.0, B3[:],
                                               OP.mult, OP.add)
                nc.tensor.matmul(ppool[:], x3[:], POOLM[:, t, :],
                                 start=(t == 0), stop=(t == NTILE - 1))
            edge_phase(3, l3_out)

            # classifier partial before AllReduce: [64, 16] = pooled^T @ Wc
            pp = spool.tile([128, G], bf, tag="pp")
            nc.vector.tensor_copy(pp[:], ppool[:])
            psl = psZ.tile([G, 16], f32, tag="z")
            nc.tensor.matmul(psl[:], pp[:], WC[:], start=True, stop=True)
            lgp = spool.tile([G, 16], f32, tag="lgp")
            nc.vector.tensor_copy(lgp[:], psl[:])
            nc.sync.dma_start(PIN[:], lgp[:])
            nc.gpsimd.collective_compute(
                "AllReduce", bass.mybir.AluOpType.add,
                replica_groups=RG,
                ins=[PIN.opt()], outs=[POUT.opt()])
            arf = spool.tile([G, 16], f32, tag="arf")
            nc.sync.dma_start(arf[:], POUT[:])
            lg = spool.tile([G, NC_CLS], f32, tag="lg2")
            nc.vector.tensor_scalar(lg[:], arf[:, 0:NC_CLS], ICNT[:], None,
                                    OP.mult, OP.bypass)
            nc.vector.tensor_tensor(lg[:], lg[:], BC[:], OP.add)
            # log_softmax over free dim (10)
            mx = spool.tile([G, 1], f32, tag="mx")
            nc.vector.tensor_reduce(mx[:], lg[:], mybir.AxisListType.X, OP.max)
            sh = spool.tile([G, NC_CLS], f32, tag="sh")
            nc.vector.tensor_scalar(sh[:], lg[:], mx[:], None, OP.subtract, OP.bypass)
            ex = spool.tile([G, NC_CLS], f32, tag="ex")
            nc.scalar.activation(ex[:], sh[:], AT.Exp)
            sm = spool.tile([G, 1], f32, tag="sm")
            nc.vector.tensor_reduce(sm[:], ex[:], mybir.AxisListType.X, OP.add)
            nc.scalar.activation(sm[:], sm[:], AT.Ln)
            lp = spool.tile([G, NC_CLS], f32, tag="lp")
            nc.vector.tensor_scalar(lp[:], sh[:], sm[:], None, OP.subtract, OP.bypass)
            nc.sync.dma_start(d_out[0], lg[:])
            nc.sync.dma_start(d_out[1], lp[:])

    nc.compile()
    return nc


_CACHE = {}


def kernel(**inputs):
    from concourse.bass_utils import run_bass_kernel_spmd

    x = np.asarray(inputs["x"], np.float32)
    edge_index = np.asarray(inputs["edge_index"], np.int64)
    batch = np.asarray(inputs["batch"], np.int64)
    weights = [np.asarray(inputs[k], np.float32) for k in
               ["W1", "a1s", "a1d", "b1", "W2", "a2s", "a2d", "b2",
                "W3", "a3s", "a3d", "b3", "Wc", "bc"]]

    per_core, nch, totc = _build_host_data(x, edge_index, batch, weights)

    key = (tuple(nch[0]), tuple(nch[1]))
    if key not in _CACHE:
        _CACHE[key] = _build_bass(nch, totc)
    nc = _CACHE[key]

    in_maps = [per_core[k] for k in range(N_CORES)]
    last_err = None
    out = None
    for attempt in range(10):
        try:
            res = run_bass_kernel_spmd(nc, in_maps, core_ids=list(range(N_CORES)))
            out = res.results[0]["out"]
            if np.all(np.isfinite(out)):
                return (np.asarray(out[0], np.float32),
                        np.asarray(out[1], np.float32))
        except Exception as e:  # transient NRT/device failures: retry
            last_err = e
            import time
            time.sleep(min(2 + 2 * attempt, 10))
    if out is not None:
        return np.asarray(out[0], np.float32), np.asarray(out[1], np.float32)
    raise last_err


if __name__ == "__main__":
    sys.path.insert(0, '/root/problem')
    import reference
    ins = {k: np.asarray(v) for k, v in reference.setup_inputs().items()}
    got = kernel(**ins)
    exp = reference.reference(**ins)
    for g_, e_ in zip(got, exp):
        e_ = np.asarray(e_)
        err = np.abs(g_ - e_).max() / (np.abs(e_).max() + 1e-9)
        print("rel err:", err)
